# revision 27
# baseline (speedup 1.0000x reference)
"""Trainium2 Bass kernel for 3-layer GATv2 (edge features) + global pool + MLP.

Distribution: edges sharded by destination node across 8 cores (dst-sorted,
window-aligned). Node features transformed locally per shard; per-layer
all-gather of the source-side transform table; per-dst segment softmax and
message aggregation done fully on-core via one-hot matmuls in PSUM.

kernel(**inputs) takes FULL inputs (as produced by the reference
setup_inputs) and returns the FULL [G, 1] output.
"""

import numpy as np

import concourse.bass as bass
import concourse.mybir as mybir
import concourse.tile as tile
from concourse import bacc, bass_utils
from concourse.bass import IndirectOffsetOnAxis
from concourse.masks import make_identity

F32 = mybir.dt.float32
I32 = mybir.dt.int32
AF = mybir.ActivationFunctionType
OP = mybir.AluOpType

# ---------------- problem constants (hardcoded per the task contract) -------
N, E, F, ED, HID, HEADS, G = 50000, 500000, 128, 6, 64, 4, 256
HC = HEADS * HID  # 256
NEG_SLOPE = 0.2
NCORES = 8
NPC = N // NCORES      # 6250 nodes per core
WIN = 128              # dst-window size (nodes)
TILE_E = 128           # edges per tile
NW = (NPC + WIN - 1) // WIN  # 49 windows per core
GB = 4                 # tiles per gather/one-hot group


# ---------------------------- host-side prep --------------------------------

def _host_prep(edge_index, edge_attr):
    src = np.asarray(edge_index[0]).astype(np.int64)
    dst = np.asarray(edge_index[1]).astype(np.int64)
    order = np.argsort(dst, kind="stable")
    s_src, s_dst = src[order], dst[order]
    s_ea = np.asarray(edge_attr, dtype=np.float32)[order]

    core = s_dst // NPC
    rel = s_dst - core * NPC
    wid = rel // WIN
    counts = np.zeros((NCORES, NW), dtype=np.int64)
    np.add.at(counts, (core, wid), 1)
    tiles_per_window = np.maximum(1, (counts + TILE_E - 1) // TILE_E).max(axis=0)
    T_total = int(tiles_per_window.sum())

    src_pad = np.zeros((NCORES, T_total * TILE_E), dtype=np.int32)
    dstrel_pad = np.full((NCORES, T_total * TILE_E), -1.0, dtype=np.float32)
    ea_pad = np.zeros((NCORES, T_total * TILE_E, ED), dtype=np.float32)

    flat_start = np.concatenate([[0], np.cumsum(counts.reshape(-1))[:-1]])
    start = flat_start.reshape(NCORES, NW)
    tstart = np.concatenate([[0], np.cumsum(tiles_per_window * TILE_E)[:-1]])

    for c in range(NCORES):
        for w in range(NW):
            k = int(counts[c, w])
            s0 = int(start[c, w])
            p0 = int(tstart[w])
            src_pad[c, p0:p0 + k] = s_src[s0:s0 + k]
            dstrel_pad[c, p0:p0 + k] = (
                s_dst[s0:s0 + k] - c * NPC - w * WIN).astype(np.float32)
            ea_pad[c, p0:p0 + k] = s_ea[s0:s0 + k]

    return src_pad, dstrel_pad, ea_pad, [int(t) for t in tiles_per_window], T_total


def _att_blockdiag(att):
    H, C = att.shape
    bd = np.zeros((H * C, H), dtype=np.float32)
    for h in range(H):
        bd[h * C:(h + 1) * C, h] = att[h]
    return bd


def _esel_aug(H, c_out):
    """[H, c_out + H]: head selector for message scaling + identity columns."""
    C = c_out // H
    m = np.zeros((H, c_out + H), dtype=np.float32)
    for h in range(H):
        m[h, h * C:(h + 1) * C] = 1.0
        m[h, c_out + h] = 1.0
    return m


def _khalf_pack(w):
    """[K, M] with K = k*128 -> [128, k*M] (row-halves side by side)."""
    K, M = w.shape
    assert K % 128 == 0
    k = K // 128
    return np.concatenate([w[q * 128:(q + 1) * 128] for q in range(k)], axis=1)


# ---------------------------- kernel builder --------------------------------

class _Cfg:
    """Sizes for the kernel builder (full problem or small sim config)."""

    def __init__(self, n, npc, nw, tiles_per_window, ncores, g):
        self.n = n
        self.npc = npc
        self.nw = nw
        self.tpw = tiles_per_window
        self.T = sum(tiles_per_window)
        self.ncores = ncores
        self.g = g
        # per layer: (k_in, c_out, H)
        self.layers = [(F, HC, HEADS), (HC, HC, HEADS), (HC, HID, 1)]


def _build(cfg: _Cfg):
    nc = bacc.Bacc(
        "TRN2", target_bir_lowering=False, debug=False,
        enable_asserts=False, num_devices=cfg.ncores,
    )

    npc, nw, tpw, T = cfg.npc, cfg.nw, cfg.tpw, cfg.T
    n_nodes, g = cfg.n, cfg.g
    tstart = np.concatenate([[0], np.cumsum(np.asarray(tpw))[:-1]]).astype(int)

    # ---- I/O declarations ----
    def din(name, shape, dt=F32):
        return nc.dram_tensor(name, list(shape), dt, kind="ExternalInput").ap()

    xT_d = din("xT", [128, npc])
    src_d = din("srcidx", [128, cfg.T], I32)
    drel_d = din("dstrel", [128, cfg.T])
    ea_d = din("eaT", [ED, cfg.T * TILE_E])
    batch_d = din("batchw", [128, nw])
    wcat_d = [
        din("wcat1", [128, 2 * HC]),
        din("wcat2", [128, 2 * 2 * HC]),
        din("wcat3", [128, 2 * 2 * HID]),
    ]
    wedge_d = [din("wedge1", [ED, HC]), din("wedge2", [ED, HC]),
               din("wedge3", [ED, HID])]
    attbd_d = [din("attbd1", [128, 2 * HEADS]), din("attbd2", [128, 2 * HEADS]),
               din("attbd3", [HID, 1])]
    esel_d = [din("esel1", [HEADS, HC + HEADS]), din("esel2", [HEADS, HC + HEADS]),
              din("esel3", [1, HID + 1])]
    bias_d = [din("bias1", [1, HC]), din("bias2", [1, HC]), din("bias3", [1, HID])]
    fc1w_d = din("fc1w", [HID, HID])
    fc1b_d = din("fc1b", [HID, 1])
    outw_d = din("outw", [HID, 1])
    outb_d = din("outb", [1, 1])
    out_d = nc.dram_tensor("out", [1, g], F32, kind="ExternalOutput").ap()

    with tile.TileContext(nc) as tc:
        res_pool_cm = tc.tile_pool(name="resident", bufs=1)
        res_pool = res_pool_cm.__enter__()

        def rtile(shape, dtype, name):
            return res_pool.tile(shape, dtype, tag=name, name=name)

        # ---------------- resident SBUF tensors ----------------
        hT_sb = rtile([128, 2 * npc], F32, "hT")
        xd_sb = rtile([128, nw * HC], F32, "xd")
        h3_sb = rtile([128, nw * HID], F32, "h3")
        src_sb = rtile([128, T], I32, "srcsb")
        drel_sb = rtile([128, T], F32, "drelsb")
        batch_sb = rtile([128, nw], F32, "batchsb")
        wcat_sb = [rtile([128, d.shape[1]], F32, f"wcat{i}")
                   for i, d in enumerate(wcat_d)]
        wedge_sb = [rtile([ED, d.shape[1]], F32, f"wedge{i}")
                    for i, d in enumerate(wedge_d)]
        attbd_sb = [rtile(list(d.shape), F32, f"attbd{i}")
                    for i, d in enumerate(attbd_d)]
        esel_sb = [rtile(list(d.shape), F32, f"esel{i}")
                   for i, d in enumerate(esel_d)]
        bias_sb = [rtile([128, d.shape[1]], F32, f"biasm{i}")
                   for i, d in enumerate(bias_d)]
        fc1w_sb = rtile([HID, HID], F32, "fc1wsb")
        fc1b_sb = rtile([HID, 1], F32, "fc1bsb")
        outw_sb = rtile([HID, 1], F32, "outwsb")
        outb_sb = rtile([1, 1], F32, "outbsb")
        ident = rtile([128, 128], F32, "ident")
        iota_mat4 = rtile([128, GB * 128], F32, "iotamat4")
        giota = rtile([128, g], F32, "giota")

        # loads of resident data
        nc.gpsimd.memset(xd_sb[:, :], 0.0)
        nc.gpsimd.memset(hT_sb[:, :], 0.0)
        nc.sync.dma_start(hT_sb[:, :npc], xT_d[:, :])
        nc.sync.dma_start(src_sb[:, :], src_d[:, :])
        nc.sync.dma_start(drel_sb[:, :], drel_d[:, :])
        nc.sync.dma_start(batch_sb[:, :], batch_d[:, :])
        for sb, d in zip(wcat_sb + wedge_sb + attbd_sb + esel_sb,
                         wcat_d + wedge_d + attbd_d + esel_d):
            nc.sync.dma_start(sb[:, :], d[:, :])
        for sb, d in zip([fc1w_sb, fc1b_sb, outw_sb, outb_sb],
                         [fc1w_d, fc1b_d, outw_d, outb_d]):
            nc.sync.dma_start(sb[:, :], d[:, :])
        for sb, d in zip(bias_sb, bias_d):
            # broadcast [1, c] -> [128, c]
            nc.sync.dma_start(sb[:, :], d[0:1, :].to_broadcast([128, d.shape[1]]))

        # consts
        make_identity(nc, ident[:, :])
        im_i = rtile([128, GB * 128], I32, "im_i")
        gi_i = rtile([128, g], I32, "gi_i")
        nc.gpsimd.iota(im_i[:, :].rearrange("p (a b) -> p a b", a=GB),
                       pattern=[[0, GB], [1, 128]], base=0, channel_multiplier=0)
        nc.gpsimd.iota(gi_i[:, :], pattern=[[1, g]], base=0, channel_multiplier=0)
        nc.vector.tensor_copy(iota_mat4[:, :], im_i[:, :])
        nc.vector.tensor_copy(giota[:, :], gi_i[:, :])

        # ---------------- DRAM scratch ----------------
        with tc.tile_pool(name="dram", bufs=1, space="DRAM") as dpool:
            xs_shard_big = dpool.tile([npc, HC + HEADS], F32)
            xs_full_big = dpool.tile([n_nodes, HC + HEADS], F32)
            xs_shard_small = dpool.tile([npc, HID + 1], F32)
            xs_full_small = dpool.tile([n_nodes, HID + 1], F32)
            pool_in = dpool.tile([HID, g], F32)
            pool_out = dpool.tile([HID, g], F32)

            for li, (k_in, c_out, H) in enumerate(cfg.layers):
                khalves = k_in // 128
                chalves = (c_out + 127) // 128
                CA = c_out + H  # augmented width
                cw0 = min(128, c_out)
                xs_shard = xs_shard_big if c_out == HC else xs_shard_small
                xs_full = xs_full_big if c_out == HC else xs_full_small

                # ---------- dense phase: xd shard + xs shard ----------
                with tc.tile_pool(name=f"dps{li}", bufs=2, space="PSUM") as psd_p, \
                     tc.tile_pool(name=f"dsb{li}", bufs=3) as dsb_p:
                    for w in range(nw):
                        nn_ = min(WIN, npc - w * WIN)
                        psd = psd_p.tile([128, 2 * c_out], F32, tag="psd")
                        for q in range(khalves):
                            lhsT = hT_sb[:, q * npc + w * WIN:
                                         q * npc + w * WIN + nn_]
                            rhs = wcat_sb[li][:, q * 2 * c_out:(q + 1) * 2 * c_out]
                            nc.tensor.matmul(psd[:nn_, :], lhsT, rhs,
                                             start=(q == 0), stop=(q == khalves - 1))
                        nc.vector.tensor_copy(
                            xd_sb[:nn_, w * c_out:(w + 1) * c_out], psd[:nn_, :c_out])
                        xs_stage = dsb_p.tile([128, CA], F32, tag="xs_stage")
                        nc.scalar.activation(xs_stage[:nn_, :c_out],
                                             psd[:nn_, c_out:], AF.Copy)
                        nc.vector.memset(xs_stage[:, c_out:], 1.0)
                        nc.sync.dma_start(
                            xs_shard[w * WIN: w * WIN + nn_, :], xs_stage[:nn_, :])

                # ---------- all-gather xs ----------
                if cfg.ncores == 1:
                    # timeline-sim build: stand in for the collective
                    nc.sync.dma_start(xs_full[:npc, :], xs_shard[:, :])
                else:
                    nc.gpsimd.collective_compute(
                        "AllGather", OP.bypass,
                        replica_groups=[list(range(cfg.ncores))],
                        ins=[xs_shard.opt()], outs=[xs_full.opt()],
                    )

                # ---------- edge phase ----------
                bank_w = 512  # fp32 elems per PSUM bank row
                with tc.tile_pool(name=f"eg{li}", bufs=3) as g_p, \
                     tc.tile_pool(name=f"ea{li}", bufs=2) as ea_p, \
                     tc.tile_pool(name=f"oh{li}", bufs=3) as oh_p, \
                     tc.tile_pool(name=f"zt{li}", bufs=2) as zt_p, \
                     tc.tile_pool(name=f"ms{li}", bufs=3) as ms_p, \
                     tc.tile_pool(name=f"et{li}", bufs=2) as et_p, \
                     tc.tile_pool(name=f"fin{li}", bufs=2) as fin_p, \
                     tc.tile_pool(name=f"ptt{li}", bufs=2, space="PSUM") as ptt_p, \
                     tc.tile_pool(name=f"pst{li}", bufs=2, space="PSUM") as pst_p, \
                     tc.tile_pool(name=f"psA{li}", bufs=2, space="PSUM") as psA_p, \
                     tc.tile_pool(name=f"pac{li}", bufs=2, space="PSUM") as pac_p:
                    for w in range(nw):
                        nn_ = min(WIN, npc - w * WIN)
                        ntile = tpw[w]
                        t0w = int(tstart[w])
                        acc = pac_p.tile([128, CA], F32, tag="acc")
                        # one DMA for the whole window's edge attrs [6, nt*128]
                        eaW = ea_p.tile([ED, ntile * TILE_E], F32, tag="eaW")
                        nc.sync.dma_start(
                            eaW[:, :ntile * TILE_E],
                            ea_d[:, t0w * TILE_E:(t0w + ntile) * TILE_E])
                        ti = 0
                        for g0 in range(0, ntile, GB):
                            gs = min(GB, ntile - g0)
                            ew = gs * TILE_E
                            t = t0w + g0
                            # gather [128, gs, CA] (one indirect DMA per tile —
                            # multi-column offset APs silently no-op on HW)
                            xs_g = g_p.tile([128, GB, CA], F32, tag="xs_g")
                            for k in range(gs):
                                nc.gpsimd.indirect_dma_start(
                                    out=xs_g[:, k, :], out_offset=None,
                                    in_=xs_full[:, :],
                                    in_offset=IndirectOffsetOnAxis(
                                        ap=src_sb[:, t + k:t + k + 1], axis=0),
                                )
                            # batched one-hot S [128e, gs, 128j]
                            S4 = oh_p.tile([128, GB * 128], F32, tag="S4")
                            nc.vector.tensor_tensor(
                                S4[:, :].rearrange("p (a b) -> p a b", a=GB)[:, :gs, :],
                                drel_sb[:, t:t + gs].to_broadcast([128, gs, 128]),
                                iota_mat4[:, :].rearrange(
                                    "p (a b) -> p a b", a=GB)[:, :gs, :],
                                op=OP.is_equal)
                            # S_T blocks via PE transpose + ACT copy to SBUF
                            # (one PSUM tile per transpose: start=True zeroes
                            # the whole bank on HW, so blocks can't share one)
                            ST4 = oh_p.tile([128, GB * 128], F32, tag="ST4")
                            for k in range(gs):
                                stp = pst_p.tile([128, 128], F32, tag="stp")
                                nc.tensor.transpose(
                                    stp[:, :], S4[:, k * 128:(k + 1) * 128],
                                    ident[:, :])
                                nc.scalar.activation(
                                    ST4[:, k * 128:(k + 1) * 128], stp[:, :],
                                    AF.Copy)
                            # per-c-half tT (one PSUM bank each) -> finer
                            # ACT/DVE pipelining; logits accumulate over halves
                            lg = psA_p.tile([H, GB * TILE_E], F32, tag="psA")
                            zT = zt_p.tile([cw0, chalves * GB * TILE_E], F32,
                                           tag="zT")
                            for q in range(chalves):
                                cw = min(128, c_out - q * 128)
                                tT = ptt_p.tile([cw0, bank_w], F32, tag="tT")
                                sl = tT[:cw, :ew]
                                nc.tensor.matmul(
                                    sl, wedge_sb[li][:, q * 128:q * 128 + cw],
                                    eaW[:, g0 * TILE_E: g0 * TILE_E + ew],
                                    start=True, stop=False)
                                nc.tensor.matmul(
                                    sl,
                                    xd_sb[:, w * c_out + q * 128:
                                          w * c_out + q * 128 + cw],
                                    ST4[:, :ew], start=False, stop=False)
                                for k in range(gs):
                                    nc.tensor.matmul(
                                        tT[:cw, k * TILE_E:(k + 1) * TILE_E],
                                        xs_g[:, k, q * 128:q * 128 + cw],
                                        ident[:, :], is_transpose=True,
                                        start=False, stop=(k == gs - 1))
                                # leaky: z = 0.6 t + 0.4 |t| for this half
                                abT = zt_p.tile([cw0, GB * TILE_E], F32,
                                                tag="abT")
                                nc.scalar.activation(
                                    abT[:cw, :ew], tT[:cw, :ew],
                                    AF.Abs, scale=(1.0 - NEG_SLOPE) / 2)
                                zsl = zT[:cw, q * GB * TILE_E:
                                         q * GB * TILE_E + ew]
                                nc.vector.scalar_tensor_tensor(
                                    zsl, tT[:cw, :ew],
                                    (1.0 + NEG_SLOPE) / 2, abT[:cw, :ew],
                                    op0=OP.mult, op1=OP.add)
                                nc.tensor.matmul(
                                    lg[:, :ew],
                                    attbd_sb[li][:cw, q * H:(q + 1) * H],
                                    zsl, start=(q == 0),
                                    stop=(q == chalves - 1))
                            eT = et_p.tile([H, GB * TILE_E], F32, tag="eT")
                            nc.scalar.activation(eT[:, :ew], lg[:, :ew], AF.Exp)
                            for k in range(gs):
                                er = psA_p.tile([128, CA], F32, tag="psA")
                                nc.tensor.matmul(
                                    er[:, :], eT[:, k * TILE_E:(k + 1) * TILE_E],
                                    esel_sb[li][:, :], start=True, stop=True)
                                msg = ms_p.tile([128, CA], F32, tag="msg")
                                nc.vector.tensor_tensor(
                                    msg[:, :], xs_g[:, k, :], er[:, :],
                                    op=OP.mult)
                                nc.tensor.matmul(
                                    acc[:, :], S4[:, k * 128:(k + 1) * 128],
                                    msg[:, :], start=(ti == 0),
                                    stop=(ti == ntile - 1))
                                ti += 1
                        # ---- window finalize ----
                        C = c_out // H
                        dn = fin_p.tile([128, H], F32, tag="dn")
                        nc.vector.tensor_scalar_add(dn[:, :], acc[:, c_out:], 1e-16)
                        rcp = fin_p.tile([128, H], F32, tag="rcp")
                        nc.vector.reciprocal(rcp[:, :], dn[:, :])
                        vv = fin_p.tile([128, c_out], F32, tag="vv")
                        for h in range(H):
                            nc.vector.scalar_tensor_tensor(
                                vv[:, h * C:(h + 1) * C],
                                acc[:, h * C:(h + 1) * C],
                                rcp[:, h:h + 1],
                                bias_sb[li][:, h * C:(h + 1) * C],
                                op0=OP.mult, op1=OP.add)
                        # elu(v) = max(v,0) + exp(min(v,0)) - 1
                        mn = fin_p.tile([128, c_out], F32, tag="mn")
                        nc.vector.tensor_scalar_min(mn[:, :], vv[:, :], 0.0)
                        em = fin_p.tile([128, c_out], F32, tag="em")
                        nc.scalar.activation(em[:, :], mn[:, :], AF.Exp)
                        rp = fin_p.tile([128, c_out], F32, tag="rp")
                        nc.vector.tensor_scalar_max(rp[:, :], vv[:, :], 0.0)
                        hn = fin_p.tile([128, c_out], F32, tag="hn")
                        nc.vector.scalar_tensor_tensor(
                            hn[:, :], em[:, :], -1.0, rp[:, :],
                            op0=OP.add, op1=OP.add)
                        if li < 2:
                            for q in range(chalves):
                                htp = psA_p.tile([128, 128], F32, tag="psA")
                                nc.tensor.transpose(
                                    htp[:, :], hn[:, q * 128:(q + 1) * 128],
                                    ident[:, :])
                                nc.scalar.activation(
                                    hT_sb[:, q * npc + w * WIN:
                                          q * npc + w * WIN + nn_],
                                    htp[:, :nn_], AF.Copy)
                        else:
                            nc.scalar.activation(
                                h3_sb[:, w * HID:(w + 1) * HID], hn[:, :], AF.Copy)

            # ---------------- pooling ----------------
            with tc.tile_pool(name="poolp", bufs=2, space="PSUM") as pp_p, \
                 tc.tile_pool(name="pools", bufs=3) as ps_p:
                gps = pp_p.tile([HID, g], F32, tag="gps")
                for w in range(nw):
                    Sg = ps_p.tile([128, g], F32, tag="Sg")
                    nc.vector.tensor_tensor(
                        Sg[:, :], batch_sb[:, w:w + 1].to_broadcast([128, g]),
                        giota[:, :], op=OP.is_equal)
                    nc.tensor.matmul(gps[:, :], h3_sb[:, w * HID:(w + 1) * HID],
                                     Sg[:, :], start=(w == 0), stop=(w == nw - 1))
                gsb = ps_p.tile([HID, g], F32, tag="gsb")
                nc.vector.tensor_copy(gsb[:, :], gps[:, :])
                nc.sync.dma_start(pool_in[:, :], gsb[:, :])
                if cfg.ncores == 1:
                    nc.sync.dma_start(pool_out[:, :], pool_in[:, :])
                else:
                    nc.gpsimd.collective_compute(
                        "AllReduce", OP.add,
                        replica_groups=[list(range(cfg.ncores))],
                        ins=[pool_in.opt()], outs=[pool_out.opt()],
                    )
                pooled = ps_p.tile([HID, g], F32, tag="pooled")
                nc.sync.dma_start(pooled[:, :], pool_out[:, :])
                # fc1 + elu
                yps = pp_p.tile([HID, g], F32, tag="yps")
                nc.tensor.matmul(yps[:, :], fc1w_sb[:, :], pooled[:, :],
                                 start=True, stop=True)
                v1 = ps_p.tile([HID, g], F32, tag="v1")
                nc.vector.tensor_scalar_add(v1[:, :], yps[:, :], fc1b_sb[:, 0:1])
                mn1 = ps_p.tile([HID, g], F32, tag="mn1")
                nc.vector.tensor_scalar_min(mn1[:, :], v1[:, :], 0.0)
                em1 = ps_p.tile([HID, g], F32, tag="em1")
                nc.scalar.activation(em1[:, :], mn1[:, :], AF.Exp)
                rp1 = ps_p.tile([HID, g], F32, tag="rp1")
                nc.vector.tensor_scalar_max(rp1[:, :], v1[:, :], 0.0)
                y1 = ps_p.tile([HID, g], F32, tag="y1")
                nc.vector.scalar_tensor_tensor(
                    y1[:, :], em1[:, :], -1.0, rp1[:, :], op0=OP.add, op1=OP.add)
                # output layer
                ops_ = pp_p.tile([1, g], F32, tag="ops")
                nc.tensor.matmul(ops_[:, :], outw_sb[:, :], y1[:, :],
                                 start=True, stop=True)
                ores = ps_p.tile([1, g], F32, tag="ores")
                nc.vector.tensor_scalar_add(ores[:, :], ops_[:, :], outb_sb[0:1, 0:1])
                nc.sync.dma_start(out_d[:, :], ores[:, :])

        res_pool_cm.__exit__(None, None, None)

    nc.compile()
    return nc


# ---------------------------- public entry ----------------------------------

_CACHE = {}


def _prepare(inputs):
    src_pad, dstrel_pad, ea_pad, tpw, T = _host_prep(
        inputs["edge_index"], inputs["edge_attr"])

    x = np.ascontiguousarray(np.asarray(inputs["x"], np.float32))
    batch = np.asarray(inputs["batch"]).astype(np.int64)

    def f32(a):
        return np.ascontiguousarray(np.asarray(a, np.float32))

    wcat1 = np.concatenate([f32(inputs["w_dst1"]), f32(inputs["w_src1"])], axis=1)
    wcat2 = _khalf_pack(
        np.concatenate([f32(inputs["w_dst2"]), f32(inputs["w_src2"])], axis=1))
    wcat3 = _khalf_pack(
        np.concatenate([f32(inputs["w_dst3"]), f32(inputs["w_src3"])], axis=1))
    attbd1 = _khalf_pack(_att_blockdiag(f32(inputs["att1"])))
    attbd2 = _khalf_pack(_att_blockdiag(f32(inputs["att2"])))
    attbd3 = _att_blockdiag(f32(inputs["att3"]))  # [64, 1]

    shared = {
        "wcat1": wcat1, "wcat2": wcat2, "wcat3": wcat3,
        "wedge1": f32(inputs["w_edge1"]), "wedge2": f32(inputs["w_edge2"]),
        "wedge3": f32(inputs["w_edge3"]),
        "attbd1": attbd1, "attbd2": attbd2, "attbd3": attbd3,
        "esel1": _esel_aug(HEADS, HC), "esel2": _esel_aug(HEADS, HC),
        "esel3": _esel_aug(1, HID),
        "bias1": f32(inputs["b1"]).reshape(1, HC),
        "bias2": f32(inputs["b2"]).reshape(1, HC),
        "bias3": f32(inputs["b3"]).reshape(1, HID),
        "fc1w": f32(inputs["fc1_w"]), "fc1b": f32(inputs["fc1_b"]).reshape(HID, 1),
        "outw": f32(inputs["out_w"]), "outb": f32(inputs["out_b"]).reshape(1, 1),
    }

    in_maps = []
    for c in range(NCORES):
        xs = x[c * NPC:(c + 1) * NPC]  # [NPC, 128]
        xT = np.ascontiguousarray(xs.T)  # [128, NPC]
        srcT = np.ascontiguousarray(
            src_pad[c].reshape(T, TILE_E).T).astype(np.int32)  # [128, T]
        drelT = np.ascontiguousarray(dstrel_pad[c].reshape(T, TILE_E).T)
        eaT = np.ascontiguousarray(ea_pad[c].T)  # [6, T*128]
        bw = np.full((128, NW), -1.0, np.float32)
        bs = batch[c * NPC:(c + 1) * NPC].astype(np.float32)
        for w in range(NW):
            nn_ = min(WIN, NPC - w * WIN)
            bw[:nn_, w] = bs[w * WIN: w * WIN + nn_]
        m = {"xT": xT, "srcidx": srcT, "dstrel": drelT, "eaT": eaT, "batchw": bw}
        m.update(shared)
        in_maps.append(m)
    return in_maps, tpw, T


LAST_RESULT = None


def kernel(**inputs) -> np.ndarray:
    global LAST_RESULT
    import os
    in_maps, tpw, T = _prepare(inputs)
    key = (T, tuple(tpw))
    if key not in _CACHE:
        cfg = _Cfg(N, NPC, NW, tpw, NCORES, G)
        _CACHE[key] = _build(cfg)
    nc = _CACHE[key]
    trace = os.environ.get("GAT_TRACE", "") == "1"
    res = bass_utils.run_bass_kernel_spmd(
        nc, in_maps, core_ids=list(range(NCORES)), trace=trace)
    LAST_RESULT = res
    out = res.results[0]["out"]  # [1, G]
    return np.ascontiguousarray(out.reshape(G, 1).astype(np.float32))


# revision 29
# speedup vs baseline: 1.3148x; 1.3148x over previous
"""Trainium2 Bass kernel for 3-layer GATv2 (edge features) + global pool + MLP.

Distribution: edges sharded by destination node across 8 cores (dst-sorted,
window-aligned). Node features transformed locally per shard; per-layer
all-gather of the source-side transform table; per-dst segment softmax and
message aggregation done fully on-core via one-hot matmuls in PSUM.

kernel(**inputs) takes FULL inputs (as produced by the reference
setup_inputs) and returns the FULL [G, 1] output.
"""

import numpy as np

import concourse.bass as bass
import concourse.mybir as mybir
import concourse.tile as tile
from concourse import bacc, bass_utils
from concourse.bass import IndirectOffsetOnAxis
from concourse.masks import make_identity

F32 = mybir.dt.float32
I32 = mybir.dt.int32
AF = mybir.ActivationFunctionType
OP = mybir.AluOpType

# ---------------- problem constants (hardcoded per the task contract) -------
N, E, F, ED, HID, HEADS, G = 50000, 500000, 128, 6, 64, 4, 256
HC = HEADS * HID  # 256
NEG_SLOPE = 0.2
NCORES = 8
NPC = N // NCORES      # 6250 nodes per core
WIN = 128              # dst-window size (nodes)
TILE_E = 128           # edges per tile
NW = (NPC + WIN - 1) // WIN  # 49 windows per core
GB = 4                 # tiles per gather/one-hot group


# ---------------------------- host-side prep --------------------------------

def _host_prep(edge_index, edge_attr):
    src = np.asarray(edge_index[0]).astype(np.int64)
    dst = np.asarray(edge_index[1]).astype(np.int64)
    order = np.argsort(dst, kind="stable")
    s_src, s_dst = src[order], dst[order]
    s_ea = np.asarray(edge_attr, dtype=np.float32)[order]

    core = s_dst // NPC
    rel = s_dst - core * NPC
    wid = rel // WIN
    counts = np.zeros((NCORES, NW), dtype=np.int64)
    np.add.at(counts, (core, wid), 1)
    tiles_per_window = np.maximum(1, (counts + TILE_E - 1) // TILE_E).max(axis=0)
    T_total = int(tiles_per_window.sum())

    src_pad = np.zeros((NCORES, T_total * TILE_E), dtype=np.int32)
    dstrel_pad = np.full((NCORES, T_total * TILE_E), -1.0, dtype=np.float32)
    ea_pad = np.zeros((NCORES, T_total * TILE_E, ED), dtype=np.float32)

    flat_start = np.concatenate([[0], np.cumsum(counts.reshape(-1))[:-1]])
    start = flat_start.reshape(NCORES, NW)
    tstart = np.concatenate([[0], np.cumsum(tiles_per_window * TILE_E)[:-1]])

    for c in range(NCORES):
        for w in range(NW):
            k = int(counts[c, w])
            s0 = int(start[c, w])
            p0 = int(tstart[w])
            src_pad[c, p0:p0 + k] = s_src[s0:s0 + k]
            dstrel_pad[c, p0:p0 + k] = (
                s_dst[s0:s0 + k] - c * NPC - w * WIN).astype(np.float32)
            ea_pad[c, p0:p0 + k] = s_ea[s0:s0 + k]

    return src_pad, dstrel_pad, ea_pad, [int(t) for t in tiles_per_window], T_total


def _att_blockdiag(att):
    H, C = att.shape
    bd = np.zeros((H * C, H), dtype=np.float32)
    for h in range(H):
        bd[h * C:(h + 1) * C, h] = att[h]
    return bd


def _esel_aug(H, c_out):
    """[H, c_out + H]: head selector for message scaling + identity columns."""
    C = c_out // H
    m = np.zeros((H, c_out + H), dtype=np.float32)
    for h in range(H):
        m[h, h * C:(h + 1) * C] = 1.0
        m[h, c_out + h] = 1.0
    return m


def _khalf_pack(w):
    """[K, M] with K = k*128 -> [128, k*M] (row-halves side by side)."""
    K, M = w.shape
    assert K % 128 == 0
    k = K // 128
    return np.concatenate([w[q * 128:(q + 1) * 128] for q in range(k)], axis=1)


# ---------------------------- kernel builder --------------------------------

class _Cfg:
    """Sizes for the kernel builder (full problem or small sim config)."""

    def __init__(self, n, npc, nw, tiles_per_window, ncores, g):
        self.n = n
        self.npc = npc
        self.nw = nw
        self.tpw = tiles_per_window
        self.T = sum(tiles_per_window)
        self.ncores = ncores
        self.g = g
        # per layer: (k_in, c_out, H)
        self.layers = [(F, HC, HEADS), (HC, HC, HEADS), (HC, HID, 1)]


def _build(cfg: _Cfg):
    nc = bacc.Bacc(
        "TRN2", target_bir_lowering=False, debug=False,
        enable_asserts=False, num_devices=cfg.ncores,
    )

    npc, nw, tpw, T = cfg.npc, cfg.nw, cfg.tpw, cfg.T
    n_nodes, g = cfg.n, cfg.g
    tstart = np.concatenate([[0], np.cumsum(np.asarray(tpw))[:-1]]).astype(int)

    # ---- I/O declarations ----
    def din(name, shape, dt=F32):
        return nc.dram_tensor(name, list(shape), dt, kind="ExternalInput").ap()

    xT_d = din("xT", [128, npc])
    src_d = din("srcidx", [128, cfg.T], I32)
    drel_d = din("dstrel", [128, cfg.T])
    ea_d = din("eaT", [ED, cfg.T * TILE_E])
    batch_d = din("batchw", [128, nw])
    wcat_d = [
        din("wcat1", [128, 2 * HC]),
        din("wcat2", [128, 2 * 2 * HC]),
        din("wcat3", [128, 2 * 2 * HID]),
    ]
    wedge_d = [din("wedge1", [ED, HC]), din("wedge2", [ED, HC]),
               din("wedge3", [ED, HID])]
    attbd_d = [din("attbd1", [128, 2 * HEADS]), din("attbd2", [128, 2 * HEADS]),
               din("attbd3", [HID, 1])]
    esel_d = [din("esel1", [HEADS, HC + HEADS]), din("esel2", [HEADS, HC + HEADS]),
              din("esel3", [1, HID + 1])]
    bias_d = [din("bias1", [1, HC]), din("bias2", [1, HC]), din("bias3", [1, HID])]
    fc1w_d = din("fc1w", [HID, HID])
    fc1b_d = din("fc1b", [HID, 1])
    outw_d = din("outw", [HID, 1])
    outb_d = din("outb", [1, 1])
    out_d = nc.dram_tensor("out", [1, g], F32, kind="ExternalOutput").ap()

    with tile.TileContext(nc) as tc:
        res_pool_cm = tc.tile_pool(name="resident", bufs=1)
        res_pool = res_pool_cm.__enter__()

        def rtile(shape, dtype, name):
            return res_pool.tile(shape, dtype, tag=name, name=name)

        # ---------------- resident SBUF tensors ----------------
        hT_sb = rtile([128, 2 * npc], F32, "hT")
        xd_sb = rtile([128, nw * HC], F32, "xd")
        h3_sb = rtile([128, nw * HID], F32, "h3")
        src_sb = rtile([128, T], I32, "srcsb")
        drel_sb = rtile([128, T], F32, "drelsb")
        batch_sb = rtile([128, nw], F32, "batchsb")
        wcat_sb = [rtile([128, d.shape[1]], F32, f"wcat{i}")
                   for i, d in enumerate(wcat_d)]
        wedge_sb = [rtile([ED, d.shape[1]], F32, f"wedge{i}")
                    for i, d in enumerate(wedge_d)]
        attbd_sb = [rtile(list(d.shape), F32, f"attbd{i}")
                    for i, d in enumerate(attbd_d)]
        esel_sb = [rtile(list(d.shape), F32, f"esel{i}")
                   for i, d in enumerate(esel_d)]
        bias_sb = [rtile([128, d.shape[1]], F32, f"biasm{i}")
                   for i, d in enumerate(bias_d)]
        fc1w_sb = rtile([HID, HID], F32, "fc1wsb")
        fc1b_sb = rtile([HID, 1], F32, "fc1bsb")
        outw_sb = rtile([HID, 1], F32, "outwsb")
        outb_sb = rtile([1, 1], F32, "outbsb")
        ident = rtile([128, 128], F32, "ident")
        iota_mat4 = rtile([128, GB * 128], F32, "iotamat4")
        giota = rtile([128, g], F32, "giota")

        # loads of resident data
        nc.gpsimd.memset(xd_sb[:, :], 0.0)
        nc.gpsimd.memset(hT_sb[:, :], 0.0)
        nc.sync.dma_start(hT_sb[:, :npc], xT_d[:, :])
        nc.sync.dma_start(src_sb[:, :], src_d[:, :])
        nc.sync.dma_start(drel_sb[:, :], drel_d[:, :])
        nc.sync.dma_start(batch_sb[:, :], batch_d[:, :])
        for sb, d in zip(wcat_sb + wedge_sb + attbd_sb + esel_sb,
                         wcat_d + wedge_d + attbd_d + esel_d):
            nc.sync.dma_start(sb[:, :], d[:, :])
        for sb, d in zip([fc1w_sb, fc1b_sb, outw_sb, outb_sb],
                         [fc1w_d, fc1b_d, outw_d, outb_d]):
            nc.sync.dma_start(sb[:, :], d[:, :])
        for sb, d in zip(bias_sb, bias_d):
            # broadcast [1, c] -> [128, c]
            nc.sync.dma_start(sb[:, :], d[0:1, :].to_broadcast([128, d.shape[1]]))

        # consts
        make_identity(nc, ident[:, :])
        im_i = rtile([128, GB * 128], I32, "im_i")
        gi_i = rtile([128, g], I32, "gi_i")
        nc.gpsimd.iota(im_i[:, :].rearrange("p (a b) -> p a b", a=GB),
                       pattern=[[0, GB], [1, 128]], base=0, channel_multiplier=0)
        nc.gpsimd.iota(gi_i[:, :], pattern=[[1, g]], base=0, channel_multiplier=0)
        nc.vector.tensor_copy(iota_mat4[:, :], im_i[:, :])
        nc.vector.tensor_copy(giota[:, :], gi_i[:, :])

        # ---------------- DRAM scratch ----------------
        with tc.tile_pool(name="dram", bufs=1, space="DRAM") as dpool:
            xs_shard_big = dpool.tile([npc, HC + HEADS], F32)
            xs_full_big = dpool.tile([n_nodes, HC + HEADS], F32)
            xs_shard_small = dpool.tile([npc, HID + 1], F32)
            xs_full_small = dpool.tile([n_nodes, HID + 1], F32)
            pool_in = dpool.tile([HID, g], F32)
            pool_out = dpool.tile([HID, g], F32)

            for li, (k_in, c_out, H) in enumerate(cfg.layers):
                khalves = k_in // 128
                chalves = (c_out + 127) // 128
                CA = c_out + H  # augmented width
                cw0 = min(128, c_out)
                xs_shard = xs_shard_big if c_out == HC else xs_shard_small
                xs_full = xs_full_big if c_out == HC else xs_full_small

                # ---------- dense phase: xd shard + xs shard ----------
                with tc.tile_pool(name=f"dps{li}", bufs=2, space="PSUM") as psd_p, \
                     tc.tile_pool(name=f"dsb{li}", bufs=3) as dsb_p:
                    for w in range(nw):
                        nn_ = min(WIN, npc - w * WIN)
                        psd = psd_p.tile([128, 2 * c_out], F32, tag="psd")
                        for q in range(khalves):
                            lhsT = hT_sb[:, q * npc + w * WIN:
                                         q * npc + w * WIN + nn_]
                            rhs = wcat_sb[li][:, q * 2 * c_out:(q + 1) * 2 * c_out]
                            nc.tensor.matmul(psd[:nn_, :], lhsT, rhs,
                                             start=(q == 0), stop=(q == khalves - 1))
                        nc.vector.tensor_copy(
                            xd_sb[:nn_, w * c_out:(w + 1) * c_out], psd[:nn_, :c_out])
                        xs_stage = dsb_p.tile([128, CA], F32, tag="xs_stage")
                        nc.scalar.activation(xs_stage[:nn_, :c_out],
                                             psd[:nn_, c_out:], AF.Copy)
                        nc.vector.memset(xs_stage[:, c_out:], 1.0)
                        nc.sync.dma_start(
                            xs_shard[w * WIN: w * WIN + nn_, :], xs_stage[:nn_, :])

                # ---------- all-gather xs ----------
                if cfg.ncores == 1:
                    # timeline-sim build: stand in for the collective
                    nc.sync.dma_start(xs_full[:npc, :], xs_shard[:, :])
                else:
                    nc.gpsimd.collective_compute(
                        "AllGather", OP.bypass,
                        replica_groups=[list(range(cfg.ncores))],
                        ins=[xs_shard.opt()], outs=[xs_full.opt()],
                    )

                # ---------- edge phase ----------
                bank_w = 512  # fp32 elems per PSUM bank row
                with tc.tile_pool(name=f"eg{li}", bufs=3) as g_p, \
                     tc.tile_pool(name=f"ea{li}", bufs=2) as ea_p, \
                     tc.tile_pool(name=f"oh{li}", bufs=3) as oh_p, \
                     tc.tile_pool(name=f"zt{li}", bufs=2) as zt_p, \
                     tc.tile_pool(name=f"ms{li}", bufs=3) as ms_p, \
                     tc.tile_pool(name=f"et{li}", bufs=2) as et_p, \
                     tc.tile_pool(name=f"fin{li}", bufs=2) as fin_p, \
                     tc.tile_pool(name=f"ptt{li}", bufs=2, space="PSUM") as ptt_p, \
                     tc.tile_pool(name=f"pst{li}", bufs=2, space="PSUM") as pst_p, \
                     tc.tile_pool(name=f"psA{li}", bufs=2, space="PSUM") as psA_p, \
                     tc.tile_pool(name=f"pac{li}", bufs=2, space="PSUM") as pac_p:
                    for w in range(nw):
                        nn_ = min(WIN, npc - w * WIN)
                        ntile = tpw[w]
                        t0w = int(tstart[w])
                        acc = pac_p.tile([128, CA], F32, tag="acc")
                        # one DMA for the whole window's edge attrs [6, nt*128]
                        eaW = ea_p.tile([ED, ntile * TILE_E], F32, tag="eaW")
                        nc.sync.dma_start(
                            eaW[:, :ntile * TILE_E],
                            ea_d[:, t0w * TILE_E:(t0w + ntile) * TILE_E])
                        ti = 0
                        for g0 in range(0, ntile, GB):
                            gs = min(GB, ntile - g0)
                            ew = gs * TILE_E
                            t = t0w + g0
                            # gather [128, gs, CA] (one indirect DMA per tile —
                            # multi-column offset APs silently no-op on HW)
                            xs_g = g_p.tile([128, GB, CA], F32, tag="xs_g")
                            for k in range(gs):
                                nc.gpsimd.indirect_dma_start(
                                    out=xs_g[:, k, :], out_offset=None,
                                    in_=xs_full[:, :],
                                    in_offset=IndirectOffsetOnAxis(
                                        ap=src_sb[:, t + k:t + k + 1], axis=0),
                                )
                            # batched one-hot S [128e, gs, 128j]
                            S4 = oh_p.tile([128, GB * 128], F32, tag="S4")
                            nc.vector.tensor_tensor(
                                S4[:, :].rearrange("p (a b) -> p a b", a=GB)[:, :gs, :],
                                drel_sb[:, t:t + gs].to_broadcast([128, gs, 128]),
                                iota_mat4[:, :].rearrange(
                                    "p (a b) -> p a b", a=GB)[:, :gs, :],
                                op=OP.is_equal)
                            # S_T blocks via PE transpose + ACT copy to SBUF
                            # (one PSUM tile per transpose: start=True zeroes
                            # the whole bank on HW, so blocks can't share one)
                            ST4 = oh_p.tile([128, GB * 128], F32, tag="ST4")
                            for k in range(gs):
                                stp = pst_p.tile([128, 128], F32, tag="stp")
                                nc.tensor.transpose(
                                    stp[:, :], S4[:, k * 128:(k + 1) * 128],
                                    ident[:, :])
                                nc.scalar.activation(
                                    ST4[:, k * 128:(k + 1) * 128], stp[:, :],
                                    AF.Copy)
                            # per-c-half tT (one PSUM bank each) -> finer
                            # ACT/DVE pipelining; logits accumulate over halves
                            lg = psA_p.tile([H, GB * TILE_E], F32, tag="psA")
                            zT = zt_p.tile([cw0, chalves * GB * TILE_E], F32,
                                           tag="zT")
                            for q in range(chalves):
                                cw = min(128, c_out - q * 128)
                                tT = ptt_p.tile([cw0, bank_w], F32, tag="tT")
                                sl = tT[:cw, :ew]
                                nc.tensor.matmul(
                                    sl, wedge_sb[li][:, q * 128:q * 128 + cw],
                                    eaW[:, g0 * TILE_E: g0 * TILE_E + ew],
                                    start=True, stop=False)
                                nc.tensor.matmul(
                                    sl,
                                    xd_sb[:, w * c_out + q * 128:
                                          w * c_out + q * 128 + cw],
                                    ST4[:, :ew], start=False, stop=False)
                                for k in range(gs):
                                    nc.tensor.matmul(
                                        tT[:cw, k * TILE_E:(k + 1) * TILE_E],
                                        xs_g[:, k, q * 128:q * 128 + cw],
                                        ident[:, :], is_transpose=True,
                                        start=False, stop=(k == gs - 1))
                                # leaky: z = 0.6 t + 0.4 |t| for this half
                                abT = zt_p.tile([cw0, GB * TILE_E], F32,
                                                tag="abT")
                                nc.scalar.activation(
                                    abT[:cw, :ew], tT[:cw, :ew],
                                    AF.Abs, scale=(1.0 - NEG_SLOPE) / 2)
                                zsl = zT[:cw, q * GB * TILE_E:
                                         q * GB * TILE_E + ew]
                                nc.vector.scalar_tensor_tensor(
                                    zsl, tT[:cw, :ew],
                                    (1.0 + NEG_SLOPE) / 2, abT[:cw, :ew],
                                    op0=OP.mult, op1=OP.add)
                                nc.tensor.matmul(
                                    lg[:, :ew],
                                    attbd_sb[li][:cw, q * H:(q + 1) * H],
                                    zsl, start=(q == 0),
                                    stop=(q == chalves - 1))
                            eT = et_p.tile([H, GB * TILE_E], F32, tag="eT")
                            nc.scalar.activation(eT[:, :ew], lg[:, :ew], AF.Exp)
                            for k in range(gs):
                                er = psA_p.tile([128, CA], F32, tag="psA")
                                nc.tensor.matmul(
                                    er[:, :], eT[:, k * TILE_E:(k + 1) * TILE_E],
                                    esel_sb[li][:, :], start=True, stop=True)
                                msg = ms_p.tile([128, CA], F32, tag="msg")
                                nc.vector.tensor_tensor(
                                    msg[:, :], xs_g[:, k, :], er[:, :],
                                    op=OP.mult)
                                nc.tensor.matmul(
                                    acc[:, :], S4[:, k * 128:(k + 1) * 128],
                                    msg[:, :], start=(ti == 0),
                                    stop=(ti == ntile - 1))
                                ti += 1
                        # ---- window finalize ----
                        C = c_out // H
                        dn = fin_p.tile([128, H], F32, tag="dn")
                        nc.vector.tensor_scalar_add(dn[:, :], acc[:, c_out:], 1e-16)
                        rcp = fin_p.tile([128, H], F32, tag="rcp")
                        nc.vector.reciprocal(rcp[:, :], dn[:, :])
                        vv = fin_p.tile([128, c_out], F32, tag="vv")
                        for h in range(H):
                            nc.vector.scalar_tensor_tensor(
                                vv[:, h * C:(h + 1) * C],
                                acc[:, h * C:(h + 1) * C],
                                rcp[:, h:h + 1],
                                bias_sb[li][:, h * C:(h + 1) * C],
                                op0=OP.mult, op1=OP.add)
                        # elu(v) = max(v,0) + exp(min(v,0)) - 1
                        mn = fin_p.tile([128, c_out], F32, tag="mn")
                        nc.vector.tensor_scalar_min(mn[:, :], vv[:, :], 0.0)
                        em = fin_p.tile([128, c_out], F32, tag="em")
                        nc.scalar.activation(em[:, :], mn[:, :], AF.Exp)
                        rp = fin_p.tile([128, c_out], F32, tag="rp")
                        nc.vector.tensor_scalar_max(rp[:, :], vv[:, :], 0.0)
                        hn = fin_p.tile([128, c_out], F32, tag="hn")
                        nc.vector.scalar_tensor_tensor(
                            hn[:, :], em[:, :], -1.0, rp[:, :],
                            op0=OP.add, op1=OP.add)
                        if li < 2:
                            for q in range(chalves):
                                htp = psA_p.tile([128, 128], F32, tag="psA")
                                nc.tensor.transpose(
                                    htp[:, :], hn[:, q * 128:(q + 1) * 128],
                                    ident[:, :])
                                nc.scalar.activation(
                                    hT_sb[:, q * npc + w * WIN:
                                          q * npc + w * WIN + nn_],
                                    htp[:, :nn_], AF.Copy)
                        else:
                            nc.scalar.activation(
                                h3_sb[:, w * HID:(w + 1) * HID], hn[:, :], AF.Copy)

            # ---------------- pooling ----------------
            with tc.tile_pool(name="poolp", bufs=2, space="PSUM") as pp_p, \
                 tc.tile_pool(name="pools", bufs=3) as ps_p:
                gps = pp_p.tile([HID, g], F32, tag="gps")
                for w in range(nw):
                    Sg = ps_p.tile([128, g], F32, tag="Sg")
                    nc.vector.tensor_tensor(
                        Sg[:, :], batch_sb[:, w:w + 1].to_broadcast([128, g]),
                        giota[:, :], op=OP.is_equal)
                    nc.tensor.matmul(gps[:, :], h3_sb[:, w * HID:(w + 1) * HID],
                                     Sg[:, :], start=(w == 0), stop=(w == nw - 1))
                gsb = ps_p.tile([HID, g], F32, tag="gsb")
                nc.vector.tensor_copy(gsb[:, :], gps[:, :])
                nc.sync.dma_start(pool_in[:, :], gsb[:, :])
                if cfg.ncores == 1:
                    nc.sync.dma_start(pool_out[:, :], pool_in[:, :])
                else:
                    nc.gpsimd.collective_compute(
                        "AllReduce", OP.add,
                        replica_groups=[list(range(cfg.ncores))],
                        ins=[pool_in.opt()], outs=[pool_out.opt()],
                    )
                pooled = ps_p.tile([HID, g], F32, tag="pooled")
                nc.sync.dma_start(pooled[:, :], pool_out[:, :])
                # fc1 + elu
                yps = pp_p.tile([HID, g], F32, tag="yps")
                nc.tensor.matmul(yps[:, :], fc1w_sb[:, :], pooled[:, :],
                                 start=True, stop=True)
                v1 = ps_p.tile([HID, g], F32, tag="v1")
                nc.vector.tensor_scalar_add(v1[:, :], yps[:, :], fc1b_sb[:, 0:1])
                mn1 = ps_p.tile([HID, g], F32, tag="mn1")
                nc.vector.tensor_scalar_min(mn1[:, :], v1[:, :], 0.0)
                em1 = ps_p.tile([HID, g], F32, tag="em1")
                nc.scalar.activation(em1[:, :], mn1[:, :], AF.Exp)
                rp1 = ps_p.tile([HID, g], F32, tag="rp1")
                nc.vector.tensor_scalar_max(rp1[:, :], v1[:, :], 0.0)
                y1 = ps_p.tile([HID, g], F32, tag="y1")
                nc.vector.scalar_tensor_tensor(
                    y1[:, :], em1[:, :], -1.0, rp1[:, :], op0=OP.add, op1=OP.add)
                # output layer
                ops_ = pp_p.tile([1, g], F32, tag="ops")
                nc.tensor.matmul(ops_[:, :], outw_sb[:, :], y1[:, :],
                                 start=True, stop=True)
                ores = ps_p.tile([1, g], F32, tag="ores")
                nc.vector.tensor_scalar_add(ores[:, :], ops_[:, :], outb_sb[0:1, 0:1])
                nc.sync.dma_start(out_d[:, :], ores[:, :])

        res_pool_cm.__exit__(None, None, None)

    nc.compile()
    return nc


# ---------------------------- public entry ----------------------------------

_CACHE = {}


def _prepare(inputs):
    src_pad, dstrel_pad, ea_pad, tpw, T = _host_prep(
        inputs["edge_index"], inputs["edge_attr"])

    x = np.ascontiguousarray(np.asarray(inputs["x"], np.float32))
    batch = np.asarray(inputs["batch"]).astype(np.int64)

    def f32(a):
        return np.ascontiguousarray(np.asarray(a, np.float32))

    wcat1 = np.concatenate([f32(inputs["w_dst1"]), f32(inputs["w_src1"])], axis=1)
    wcat2 = _khalf_pack(
        np.concatenate([f32(inputs["w_dst2"]), f32(inputs["w_src2"])], axis=1))
    wcat3 = _khalf_pack(
        np.concatenate([f32(inputs["w_dst3"]), f32(inputs["w_src3"])], axis=1))
    attbd1 = _khalf_pack(_att_blockdiag(f32(inputs["att1"])))
    attbd2 = _khalf_pack(_att_blockdiag(f32(inputs["att2"])))
    attbd3 = _att_blockdiag(f32(inputs["att3"]))  # [64, 1]

    shared = {
        "wcat1": wcat1, "wcat2": wcat2, "wcat3": wcat3,
        "wedge1": f32(inputs["w_edge1"]), "wedge2": f32(inputs["w_edge2"]),
        "wedge3": f32(inputs["w_edge3"]),
        "attbd1": attbd1, "attbd2": attbd2, "attbd3": attbd3,
        "esel1": _esel_aug(HEADS, HC), "esel2": _esel_aug(HEADS, HC),
        "esel3": _esel_aug(1, HID),
        "bias1": f32(inputs["b1"]).reshape(1, HC),
        "bias2": f32(inputs["b2"]).reshape(1, HC),
        "bias3": f32(inputs["b3"]).reshape(1, HID),
        "fc1w": f32(inputs["fc1_w"]), "fc1b": f32(inputs["fc1_b"]).reshape(HID, 1),
        "outw": f32(inputs["out_w"]), "outb": f32(inputs["out_b"]).reshape(1, 1),
    }

    in_maps = []
    for c in range(NCORES):
        xs = x[c * NPC:(c + 1) * NPC]  # [NPC, 128]
        xT = np.ascontiguousarray(xs.T)  # [128, NPC]
        srcT = np.ascontiguousarray(
            src_pad[c].reshape(T, TILE_E).T).astype(np.int32)  # [128, T]
        drelT = np.ascontiguousarray(dstrel_pad[c].reshape(T, TILE_E).T)
        eaT = np.ascontiguousarray(ea_pad[c].T)  # [6, T*128]
        bw = np.full((128, NW), -1.0, np.float32)
        bs = batch[c * NPC:(c + 1) * NPC].astype(np.float32)
        for w in range(NW):
            nn_ = min(WIN, NPC - w * WIN)
            bw[:nn_, w] = bs[w * WIN: w * WIN + nn_]
        m = {"xT": xT, "srcidx": srcT, "dstrel": drelT, "eaT": eaT, "batchw": bw}
        m.update(shared)
        in_maps.append(m)
    return in_maps, tpw, T


LAST_RESULT = None


def kernel(**inputs) -> np.ndarray:
    global LAST_RESULT
    import os
    in_maps, tpw, T = _prepare(inputs)
    key = (T, tuple(tpw))
    if key not in _CACHE:
        cfg = _Cfg(N, NPC, NW, tpw, NCORES, G)
        _CACHE[key] = _build(cfg)
    nc = _CACHE[key]
    trace = os.environ.get("GAT_TRACE", "") == "1"
    res = bass_utils.run_bass_kernel_spmd(
        nc, in_maps, core_ids=list(range(NCORES)), trace=trace)
    LAST_RESULT = res
    out = res.results[0]["out"]  # [1, G]
    return np.ascontiguousarray(out.reshape(G, 1).astype(np.float32))
